# revision 1
# baseline (speedup 1.0000x reference)
"""CascadePredictor Trainium2 kernel v3.

v3 design (vs baseline):
- All row gathers use InstDMAGatherAnt (gpsimd dma_gather): one instruction per
  block-half / decode group instead of one InstDMACopy per 128 rows (~1us SWDGE
  fixed cost each). int16 index limit handled by splitting tables into
  <32768-row halves and sorting edges by source half (2 extra pad tiles/block).
- L1 aggregates raw x*dinv (128-wide rows, half the gather bytes + matmul cost),
  applying W1 after aggregation: (A @ (xW1)) == (A @ x) @ W1. Kills stage A and
  the first AllGather. b1 term folds in as rank-1 dinv*mu x b1 (zero here).
- Decode: per-core edges bucketed by (sp-half, dp-half); Q/K rows gathered from
  separate 384-wide padded tables (768B rows, 256B-multiple requirement).

  dinv[n] = 1/sqrt(indeg+1), xd = x*dinv
  A_d = sum_{e: dst=d} xd[src] + xd[d];  h = relu(dinv * (A @ W1) + dinv*mu*b1)
  hw2d = (h @ W2 + b2) * dinv                      (AllGather)
  z_d = dinv * (sum hw2d[src] + hw2d[d])
  Tq = [z@WqT*s | l0 | s0 | 0...], Tk = [z@WkT | s1 | 0...]   (2x AllGather)
  out = sigmoid(sum_h s0 + sigmoid(l1-l0)*(s1-s0) + bsum), l1 = Q'[sp].K[dp]
"""
import sys
import numpy as np

for p in ("/opt/trn_rl_repo",):
    if p not in sys.path:
        sys.path.insert(0, p)

import ml_dtypes
import concourse.bass as bass
import concourse.bacc as bacc
import concourse.tile as tile
import concourse.mybir as mybir

bf16 = ml_dtypes.bfloat16
F32 = mybir.dt.float32
BF = mybir.dt.bfloat16
I16 = mybir.dt.int16
I32 = mybir.dt.int32

import os
V3_STOP = int(os.environ.get("V3_STOP", "3"))  # 1=L1 only, 2=+L2, 3=full

NCORES = 8
P = 128
HIDDEN = 256
NH, HD = 4, 64
TWD = 384       # decode table row width (256 + 8 pad to 256B multiple)
DG = 8          # decode tiles per gather group
STB = 8         # selection-matrix tiles per is_equal


def _wrap16(idx):
    """dma_gather index layout: idx j at [j%16, j//16], replicated x8."""
    w = np.asarray(idx, np.int16).reshape(-1, 16).T
    return np.tile(w, (8, 1))


# ----------------------------------------------------------------------------
# host-side preprocessing
# ----------------------------------------------------------------------------
def build_host_data(x, edge_index, edge_index_pred,
                    W1, b1, W2, b2, in_proj_w, in_proj_b, out_proj_w, out_proj_b):
    H2PAD = HIDDEN
    N = x.shape[0]
    src = np.asarray(edge_index[0], np.int64)
    dst = np.asarray(edge_index[1], np.int64)
    sp = np.asarray(edge_index_pred[0], np.int64)
    dp = np.asarray(edge_index_pred[1], np.int64)
    E = src.shape[0]
    EP = sp.shape[0]

    NBLK = -(-N // P)
    NBLK = -(-NBLK // (2 * NCORES)) * (2 * NCORES)  # even per-core block count
    NPAD = NBLK * P
    NBC = NBLK // NCORES
    HALF = NPAD // 2
    assert HALF < 32768, "table half must fit int16 indices"

    deg = np.bincount(dst, minlength=N).astype(np.float64) + 1.0
    dinv = np.zeros(NPAD, np.float32)
    dinv[:N] = (1.0 / np.sqrt(deg)).astype(np.float32)
    musum = np.bincount(dst, weights=dinv[:N][src].astype(np.float64),
                        minlength=N) + dinv[:N]
    hasb1 = bool(np.abs(np.asarray(b1)).max() > 0)

    # --- load-balanced permutation: snake-assign nodes (sorted by indeg desc)
    indeg = (deg - 1.0).astype(np.int64)
    order = np.argsort(-indeg, kind="stable")
    snake = np.empty(N, np.int64)
    pos = np.arange(N)
    rnd, off = pos // NBLK, pos % NBLK
    fwd = (rnd % 2) == 0
    snake[fwd] = off[fwd]
    snake[~fwd] = NBLK - 1 - off[~fwd]
    blk_of = np.empty(NPAD, np.int64)
    blk_of[order] = snake[:N]
    slot_of = np.empty(NPAD, np.int64)
    counts = np.bincount(blk_of[:N], minlength=NBLK)
    assert counts.max() <= P
    o2 = np.argsort(blk_of[:N], kind="stable")
    within = np.arange(N) - np.repeat(np.concatenate([[0], np.cumsum(counts)[:-1]]), counts)
    slot_of[o2] = within
    free_blocks = np.repeat(np.arange(NBLK), P - counts)
    pad_ids = np.arange(N, NPAD)
    blk_of[pad_ids] = free_blocks[: NPAD - N]
    pad_within = []
    fc = counts.copy()
    for b in free_blocks[: NPAD - N]:
        pad_within.append(fc[b])
        fc[b] += 1
    slot_of[pad_ids] = np.array(pad_within, np.int64) if len(pad_within) else np.zeros(0, np.int64)
    perm = blk_of * P + slot_of
    assert np.array_equal(np.sort(perm), np.arange(NPAD))

    dinv_perm = np.zeros(NPAD, np.float32)
    dinv_perm[perm] = dinv
    mu_perm = np.zeros(NPAD, np.float32)
    mu_perm[perm[:N]] = musum.astype(np.float32)

    # --- edge grid: per (dst block, src half), partition-fastest slots
    pdst = perm[dst]
    psrc = perm[src]
    eblk = pdst // P
    eloc = pdst % P
    half = (psrc >= HALF).astype(np.int64)
    key = eblk * 2 + half
    cnt2 = np.bincount(key, minlength=NBLK * 2)
    T0 = int(-(-cnt2[0::2].max() // P))
    T1 = int(-(-cnt2[1::2].max() // P))
    T = T0 + T1
    eord = np.argsort(key, kind="stable")
    starts = np.concatenate([[0], np.cumsum(cnt2)[:-1]])
    epos = np.arange(E) - np.repeat(starts, cnt2)
    slot = np.where(half[eord] == 1, T0 * P, 0) + epos
    b_ = eblk[eord]
    p_ = slot % P
    t_ = slot // P
    gsrc_rel = np.zeros((NBLK, P, T), np.int16)
    dstloc = np.full((NBLK, P, T), -1.0, np.float32)
    gsrc_rel[b_, p_, t_] = (psrc[eord] - half[eord] * HALF).astype(np.int16)
    dstloc[b_, p_, t_] = eloc[eord].astype(np.float32)

    g4 = gsrc_rel.reshape(NCORES, NBC, P, T)
    d4 = dstloc.reshape(NCORES, NBC, P, T)
    aggidx_core = []
    dstloc_core = []
    for c in range(NCORES):
        cols = []
        for b in range(NBC):
            flat = g4[c, b].T.reshape(-1)        # j = t*128 + p
            cols.append(_wrap16(flat))           # [128, T*8]
        aggidx_core.append(np.concatenate(cols, axis=1))
        dstloc_core.append(np.ascontiguousarray(
            d4[c].transpose(1, 0, 2).reshape(P, NBC * T)).astype(bf16))

    # --- decode: 4 buckets by (sp half, dp half)
    EPC_raw = -(-EP // NCORES)
    pq_all, pk_all, bkt_all = [], [], []
    cnts = np.zeros((NCORES, 4), np.int64)
    for c in range(NCORES):
        lo, hi = c * EPC_raw, min((c + 1) * EPC_raw, EP)
        pq = perm[sp[lo:hi]]
        pk = perm[dp[lo:hi]]
        bkt = (pq >= HALF) * 2 + (pk >= HALF)
        pq_all.append(pq); pk_all.append(pk); bkt_all.append(bkt)
        cnts[c] = np.bincount(bkt, minlength=4)
    NT = [int(-(-cnts[:, k].max() // P)) for k in range(4)]
    NDT = sum(NT)
    qidx_core, kidx_core, inv_core = [], [], []
    for c in range(NCORES):
        pq, pk, bkt = pq_all[c], pk_all[c], bkt_all[c]
        lo = c * EPC_raw
        qf = np.zeros(NDT * P, np.int64)
        kf = np.zeros(NDT * P, np.int64)
        iv = np.full(NDT * P, -1, np.int64)
        off = 0
        for k in range(4):
            a, bb = k >> 1, k & 1
            mem = np.nonzero(bkt == k)[0]
            n = len(mem)
            qf[off:off + n] = pq[mem] - a * HALF
            kf[off:off + n] = pk[mem] - bb * HALF
            iv[off:off + n] = lo + mem
            off += NT[k] * P
        qidx_core.append(_wrap16(qf))
        kidx_core.append(_wrap16(kf))
        inv_core.append(iv)

    # --- dense weights / tables
    xp = np.zeros((NPAD, x.shape[1]), np.float32)
    xp[perm[:N]] = np.asarray(x, np.float32)[:N]
    xd_full = np.zeros((NPAD, H2PAD), bf16)
    xd_full[:, 0:x.shape[1]] = (xp * dinv_perm[:, None]).astype(bf16)

    H = HIDDEN
    Wq = in_proj_w[0:H]; Wk = in_proj_w[H:2 * H]; Wv = in_proj_w[2 * H:3 * H]
    bq = in_proj_b[0:H]; bk = in_proj_b[H:2 * H]; bv = in_proj_b[2 * H:3 * H]
    c_vec = out_proj_w.sum(axis=0)
    bsum = float(out_proj_b.sum())
    scale = 1.0 / np.sqrt(HD)
    u2 = np.stack([(Wv[h * HD:(h + 1) * HD, :] * c_vec[h * HD:(h + 1) * HD, None]).sum(0)
                   for h in range(NH)], axis=1)
    beta = np.stack([(bv[h * HD:(h + 1) * HD] * c_vec[h * HD:(h + 1) * HD]).sum()
                     for h in range(NH)])

    assert x.shape[1] == P
    dinv_cols = np.ascontiguousarray(dinv_perm.reshape(NBLK, P).T)    # [P, NBLK]
    om_cols = np.ascontiguousarray(
        (dinv_perm * mu_perm).reshape(NBLK, P).T)                     # [P, NBLK]
    iota_row = np.arange(P, dtype=np.float32)

    meta = dict(NPAD=NPAD, NBLK=NBLK, NBC=NBC, HALF=HALF, T0=T0, T1=T1, T=T,
                NDT=NDT, NT=tuple(NT), EPC_raw=EPC_raw, EP=EP, bsum=bsum,
                hasb1=hasb1)

    common = {
        "xd_full": xd_full,
        "w1": np.asarray(W1, np.float32).astype(bf16),
        "w2c": np.asarray(W2, np.float32).reshape(2, P, H).astype(bf16),
        "wqc": (np.asarray(Wq, np.float32).T * scale).reshape(2, P, H).astype(bf16),
        "wkc": np.asarray(Wk, np.float32).T.reshape(2, P, H).astype(bf16),
        "uc": u2.reshape(2, P, NH).astype(bf16),
        "b1bc": np.tile(np.asarray(b1, np.float32)[None, :], (P, 1)).astype(bf16),
        "b2r": np.asarray(b2, np.float32).reshape(1, H).astype(bf16),
        "bqr": (np.asarray(bq, np.float32) * scale).reshape(1, H).astype(bf16),
        "bkr": np.asarray(bk, np.float32).reshape(1, H).astype(bf16),
        "betar": beta.reshape(1, NH).astype(np.float32),
        "iota8": np.tile(iota_row.astype(bf16)[None, :], (P, STB)),
        "ident_bf": np.eye(P, dtype=np.float32).astype(bf16),
        "ident_f32": np.eye(P, dtype=np.float32),
    }
    in_maps = []
    for c in range(NCORES):
        m = dict(common)
        m["xdown"] = np.ascontiguousarray(xd_full[c * NBC * P:(c + 1) * NBC * P])
        m["aggidx"] = aggidx_core[c]
        m["dstloc"] = dstloc_core[c]
        m["dinv_own"] = np.ascontiguousarray(dinv_cols[:, c * NBC:(c + 1) * NBC]).astype(np.float32)
        m["om_own"] = np.ascontiguousarray(om_cols[:, c * NBC:(c + 1) * NBC]).astype(np.float32)
        m["qidx"] = qidx_core[c]
        m["kidx"] = kidx_core[c]
        in_maps.append(m)
    return in_maps, meta, inv_core


# ----------------------------------------------------------------------------
# program builder
# ----------------------------------------------------------------------------
def build_program(meta):
    NPAD, NBLK, NBC, HALF, T0, T1, T, NDT, NT = (meta[k] for k in
        ("NPAD", "NBLK", "NBC", "HALF", "T0", "T1", "T", "NDT", "NT"))
    H = HIDDEN
    hasb1 = meta["hasb1"]

    # ring must hold ~2x the largest gather's descriptors (chunk*128 + 16):
    # 49152B/16B = 3072 descs, vs default 16384B/16 = 1024 (big gathers hang).
    nc = bacc.Bacc("TRN2", target_bir_lowering=False, debug=False,
                   num_devices=NCORES, dynamic_dma_scratch_size=49152,
                   num_swdge_queues=4)

    def din(name, shape, dt):
        return nc.dram_tensor(name, shape, dt, kind="ExternalInput")

    xd_in = din("xd_full", [NPAD, H], BF)
    xdown_in = din("xdown", [NBC * P, H], BF)
    aggidx_in = din("aggidx", [P, NBC * T * 8], I16)
    dstloc_in = din("dstloc", [P, NBC * T], BF)
    dinv_own = din("dinv_own", [P, NBC], F32)
    om_own = din("om_own", [P, NBC], F32)
    w1 = din("w1", [P, H], BF)
    w2c = din("w2c", [2, P, H], BF)
    wqc = din("wqc", [2, P, H], BF)
    wkc = din("wkc", [2, P, H], BF)
    uc = din("uc", [2, P, NH], BF)
    b1bc = din("b1bc", [P, H], BF)
    b2r = din("b2r", [1, H], BF)
    bqr = din("bqr", [1, H], BF)
    bkr = din("bkr", [1, H], BF)
    betar = din("betar", [1, NH], F32)
    iota8_in = din("iota8", [P, STB * P], BF)
    identb_in = din("ident_bf", [P, P], BF)
    identf_in = din("ident_f32", [P, P], F32)
    qidx_in = din("qidx", [P, NDT * 8], I16)
    kidx_in = din("kidx", [P, NDT * 8], I16)

    out_t = nc.dram_tensor("out", [NDT * P], F32, kind="ExternalOutput")

    hw2d_shard = nc.dram_tensor("hw2d_shard", [NBC * P, H], BF, kind="Internal")
    hw2d_full = nc.dram_tensor("hw2d_full", [NPAD, H], BF, kind="Internal", addr_space="Shared")
    tq_shard = nc.dram_tensor("tq_shard", [NBC * P, TWD], BF, kind="Internal")
    tq_full = nc.dram_tensor("tq_full", [NPAD, TWD], BF, kind="Internal", addr_space="Shared")
    tk_shard = nc.dram_tensor("tk_shard", [NBC * P, TWD], BF, kind="Internal")
    tk_full = nc.dram_tensor("tk_full", [NPAD, TWD], BF, kind="Internal", addr_space="Shared")

    AG = mybir.AluOpType
    with tile.TileContext(nc) as tc:
        with tc.tile_pool(name="sb", bufs=1) as res, \
             tc.tile_pool(name="wk", bufs=3) as wk, \
             tc.tile_pool(name="gp", bufs=2) as gpool, \
             tc.tile_pool(name="st", bufs=2) as stp, \
             tc.tile_pool(name="dq", bufs=2) as dqp, \
             tc.tile_pool(name="ps", bufs=4, space="PSUM") as psp, \
             tc.tile_pool(name="pt", bufs=2, space="PSUM") as ptp:

            def load(name, src, shape, dt):
                t = res.tile(shape, dt, tag=name)
                nc.sync.dma_start(t[:], src[:])
                return t
            w1_t = load("w1", w1, [P, H], BF)
            b1bc_t = load("b1bc", b1bc, [P, H], BF)

            def load2(name, src, width, dt):
                t = res.tile([P, 2 * width], dt, tag=name)
                for k in range(2):
                    nc.sync.dma_start(t[:, k * width:(k + 1) * width], src[k])
                return t
            w2_t = load2("w2c", w2c, H, BF)
            wq_t = load2("wqc", wqc, H, BF)
            wk_t = load2("wkc", wkc, H, BF)
            uc_t = load2("uc", uc, NH, BF)
            iota8_t = load("iota8", iota8_in, [P, STB * P], BF)
            idb_t = load("idb", identb_in, [P, P], BF)
            idf_t = load("idf", identf_in, [P, P], F32)
            dinvo_t = load("dinvo", dinv_own, [P, NBC], F32)
            om_t = load("om", om_own, [P, NBC], F32)
            aggidx_t = load("aggidx", aggidx_in, [P, NBC * T * 8], I16)
            dstloc_t = load("dstloc", dstloc_in, [P, NBC * T], BF)
            qidx_t = load("qidx", qidx_in, [P, NDT * 8], I16)
            kidx_t = load("kidx", kidx_in, [P, NDT * 8], I16)

            def loadb(name, src):
                t = res.tile([P, H], BF, tag=name)
                nc.sync.dma_start(t[:], src[:].to_broadcast((P, H)))
                return t
            b2_t = loadb("b2", b2r)
            bq_t = loadb("bq", bqr)
            bk_t = loadb("bk", bkr)
            beta_b = res.tile([P, NH], F32, tag="betab")
            nc.sync.dma_start(beta_b[:], betar[:].to_broadcast((P, NH)))

            colbuf = res.tile([P, NDT], F32, tag="colbuf")
            bsum_t = res.tile([P, 1], F32, tag="bsum")
            nc.vector.memset(bsum_t[:], float(meta["bsum"]))

            # ---------------- aggregation layer template
            GCK = 5  # chunk fits per-queue ring quarter (656 <= 768)

            def agg_layer(table, selfsrc, b, finalize, narrow=False):
                W = P if narrow else H
                g = gpool.tile([P, T, H], BF, tag="g")
                gv = g[:].rearrange("p t (a b) -> p (t a) b", b=P) if narrow else g
                c0 = b * T * 8
                qn = 0
                for hh, (tbase, tcnt, tab) in enumerate(((0, T0, table[0:HALF]),
                                                         (T0, T1, table[HALF:NPAD]))):
                    nck = -(-tcnt // GCK)
                    base, extra = tcnt // nck, tcnt % nck
                    sizes = [base + (1 if i < extra else 0) for i in range(nck)]
                    if hh == 1:
                        sizes.reverse()   # balance per-queue tile loads
                    done = 0
                    for ck in sizes:
                        t0 = tbase + done
                        if narrow:
                            nc.gpsimd.dma_gather(
                                gv[:, t0:t0 + ck, :], tab[:, 0:P],
                                aggidx_t[:, c0 + t0 * 8:c0 + (t0 + ck) * 8],
                                ck * P, ck * P, P, elem_step=H, queue_num=qn % 4)
                        else:
                            nc.gpsimd.dma_gather(
                                g[:, t0:t0 + ck, :], tab,
                                aggidx_t[:, c0 + t0 * 8:c0 + (t0 + ck) * 8],
                                ck * P, ck * P, H, queue_num=qn % 4)
                        qn += 1
                        done += ck
                agg = psp.tile([P, H], F32, tag="p256", space="PSUM")
                for t0 in range(0, T, STB):
                    nb = min(STB, T - t0)
                    st8 = stp.tile([P, STB, P], BF, tag="st8")
                    nc.vector.tensor_tensor(
                        out=st8[:, 0:nb, :],
                        in0=iota8_t[:, 0:nb * P].rearrange("p (s q) -> p s q", s=nb),
                        in1=dstloc_t[:, b * T + t0:b * T + t0 + nb].to_broadcast((P, nb, P)),
                        op=AG.is_equal)
                    for j in range(nb):
                        t = t0 + j
                        nc.tensor.matmul(agg[:, 0:W], lhsT=st8[:, j, :],
                                         rhs=gv[:, t, :],
                                         start=(t == 0), stop=(t == T - 1))
                selfb = wk.tile([P, H], BF, tag="selfb")
                nc.sync.dma_start(selfb[:, 0:W], selfsrc[b * P:(b + 1) * P, 0:W])
                asum = wk.tile([P, H], F32, tag="asum")
                nc.vector.tensor_tensor(out=asum[:, 0:W], in0=agg[:, 0:W],
                                        in1=selfb[:, 0:W], op=AG.add)
                finalize(asum)

            def transposed_chunks(src_bf, tag):
                outs = []
                for k in range(2):
                    pt = ptp.tile([P, P], BF, tag="pT", space="PSUM")
                    nc.tensor.transpose(pt[:], src_bf[:, k * P:(k + 1) * P], idb_t[:])
                    sb = wk.tile([P, P], BF, tag=f"{tag}{k}")
                    nc.scalar.activation(sb[:], pt[:], mybir.ActivationFunctionType.Copy)
                    outs.append(sb)
                return outs

            # ---------------- layer 1: A = agg(xd); h1 = relu(dinv*A@W1 [+ om*b1])
            sL = nc.enter_named_scope("L1", False)
            for b in range(NBC):
                def fin1(asum, b=b):
                    abf = wk.tile([P, P], BF, tag="abf")
                    nc.scalar.activation(abf[:], asum[:, 0:P],
                                         mybir.ActivationFunctionType.Copy)
                    pt = ptp.tile([P, P], BF, tag="pT", space="PSUM")
                    nc.tensor.transpose(pt[:], abf[:], idb_t[:])
                    atb = wk.tile([P, P], BF, tag="atb")
                    nc.scalar.activation(atb[:], pt[:], mybir.ActivationFunctionType.Copy)
                    ps2 = psp.tile([P, H], F32, tag="p256", space="PSUM")
                    nc.tensor.matmul(ps2[:], lhsT=atb[:], rhs=w1_t[:],
                                     start=True, stop=True)
                    h1 = wk.tile([P, H], BF, tag="h1")
                    if hasb1:
                        ob = wk.tile([P, H], F32, tag="ob1")
                        nc.vector.tensor_tensor(
                            out=ob[:], in0=b1bc_t[:],
                            in1=om_t[:, b:b + 1].to_broadcast((P, H)), op=AG.mult)
                        t3 = wk.tile([P, H], F32, tag="t3")
                        nc.scalar.activation(t3[:], ps2[:],
                                             mybir.ActivationFunctionType.Copy,
                                             scale=dinvo_t[:, b:b + 1])
                        t4 = wk.tile([P, H], F32, tag="t4")
                        nc.vector.tensor_tensor(out=t4[:], in0=t3[:], in1=ob[:], op=AG.add)
                        nc.scalar.activation(h1[:], t4[:],
                                             mybir.ActivationFunctionType.Relu)
                    else:
                        nc.scalar.activation(h1[:], ps2[:],
                                             mybir.ActivationFunctionType.Relu,
                                             scale=dinvo_t[:, b:b + 1])
                    hts = transposed_chunks(h1, "h1T")
                    ps3 = psp.tile([P, H], F32, tag="p256", space="PSUM")
                    for k in range(2):
                        nc.tensor.matmul(ps3[:], lhsT=hts[k][:], rhs=w2_t[:, k * H:(k + 1) * H],
                                         start=(k == 0), stop=(k == 1))
                    t2 = wk.tile([P, H], F32, tag="t2")
                    nc.vector.tensor_tensor(out=t2[:], in0=ps3[:], in1=b2_t[:], op=AG.add)
                    hwb = wk.tile([P, H], BF, tag="hwb")
                    nc.scalar.activation(hwb[:], t2[:], mybir.ActivationFunctionType.Copy,
                                         scale=dinvo_t[:, b:b + 1])
                    nc.sync.dma_start(hw2d_shard[b * P:(b + 1) * P, :], hwb[:])
                agg_layer(xd_in, xdown_in, b, fin1, narrow=True)
            nc.leave_named_scope("L1", sL[0], False)

            if V3_STOP >= 2:
                sG = nc.enter_named_scope("AG2", False)
                nc.gpsimd.collective_compute(
                    "AllGather", AG.bypass, replica_groups=[list(range(NCORES))],
                    ins=[hw2d_shard[:]], outs=[hw2d_full[:]])
                nc.leave_named_scope("AG2", sG[0], False)

            # ---------------- layer 2 + decode tables
            sL = nc.enter_named_scope("L2", False)
            for b in range(NBC if V3_STOP >= 2 else 0):
                def fin2(asum, b=b):
                    zb = wk.tile([P, H], BF, tag="zb")
                    nc.scalar.activation(zb[:], asum[:], mybir.ActivationFunctionType.Copy,
                                         scale=dinvo_t[:, b:b + 1])
                    zts = transposed_chunks(zb, "zT")
                    tqb = wk.tile([P, TWD], BF, tag="tqb")
                    tkb = wk.tile([P, TWD], BF, tag="tkb")
                    psq = psp.tile([P, H], F32, tag="p256", space="PSUM")
                    for k in range(2):
                        nc.tensor.matmul(psq[:], lhsT=zts[k][:], rhs=wq_t[:, k * H:(k + 1) * H],
                                         start=(k == 0), stop=(k == 1))
                    nc.vector.tensor_tensor(out=tqb[:, 0:H], in0=psq[:], in1=bq_t[:], op=AG.add)
                    psk = psp.tile([P, H], F32, tag="p256", space="PSUM")
                    for k in range(2):
                        nc.tensor.matmul(psk[:], lhsT=zts[k][:], rhs=wk_t[:, k * H:(k + 1) * H],
                                         start=(k == 0), stop=(k == 1))
                    nc.vector.tensor_tensor(out=tkb[:, 0:H], in0=psk[:], in1=bk_t[:], op=AG.add)
                    qk = wk.tile([P, H], F32, tag="qk")
                    nc.vector.tensor_tensor(out=qk[:], in0=tqb[:, 0:H], in1=tkb[:, 0:H], op=AG.mult)
                    with nc.allow_low_precision(reason="l0 stored bf16 as before"):
                        nc.vector.tensor_reduce(out=tqb[:, H:H + NH],
                                                in_=qk[:].rearrange("p (h d) -> p h d", h=NH),
                                                axis=mybir.AxisListType.X, op=AG.add)
                    pss = ptp.tile([P, NH], F32, tag="pS", space="PSUM")
                    for k in range(2):
                        nc.tensor.matmul(pss[:], lhsT=zts[k][:], rhs=uc_t[:, k * NH:(k + 1) * NH],
                                         start=(k == 0), stop=(k == 1))
                    with nc.allow_low_precision(reason="s stored bf16 as before"):
                        nc.vector.tensor_tensor(out=tqb[:, H + NH:H + 2 * NH], in0=pss[:],
                                                in1=beta_b[:], op=AG.add)
                        nc.vector.tensor_tensor(out=tkb[:, H:H + NH], in0=pss[:],
                                                in1=beta_b[:], op=AG.add)
                    nc.vector.memset(tqb[:, H + 2 * NH:TWD], 0)
                    nc.vector.memset(tkb[:, H + NH:TWD], 0)
                    nc.sync.dma_start(tq_shard[b * P:(b + 1) * P, :], tqb[:])
                    nc.sync.dma_start(tk_shard[b * P:(b + 1) * P, :], tkb[:])
                agg_layer(hw2d_full, hw2d_shard, b, fin2)
            nc.leave_named_scope("L2", sL[0], False)

            if V3_STOP >= 3:
                sG = nc.enter_named_scope("AG3", False)
                nc.gpsimd.collective_compute(
                    "AllGather", AG.bypass, replica_groups=[list(range(NCORES))],
                    ins=[tq_shard[:]], outs=[tq_full[:]])
                nc.gpsimd.collective_compute(
                    "AllGather", AG.bypass, replica_groups=[list(range(NCORES))],
                    ins=[tk_shard[:]], outs=[tk_full[:]])
                nc.leave_named_scope("AG3", sG[0], False)
            else:
                nc.vector.memset(colbuf[:], 0)

            # ---------------- decode: 4 buckets, groups of <=DG tiles
            sD = nc.enter_named_scope("decode", False)
            gt = 0
            for k in range(4 if V3_STOP >= 3 else 0):
                a, bb = k >> 1, k & 1
                tq_half = tq_full[a * HALF:(a + 1) * HALF]
                tk_half = tk_full[bb * HALF:(bb + 1) * HALF]
                done = 0
                while done < NT[k]:
                    dgg = min(DG, NT[k] - done)
                    ni = dgg * P
                    gq = dqp.tile([P, DG, TWD], BF, tag="gq")
                    gk = dqp.tile([P, DG, TWD], BF, tag="gk")
                    h1_ = dgg // 2
                    nc.gpsimd.dma_gather(gq[:, 0:h1_, :], tq_half,
                                         qidx_t[:, gt * 8:(gt + h1_) * 8],
                                         h1_ * P, h1_ * P, TWD, queue_num=0)
                    nc.gpsimd.dma_gather(gq[:, h1_:dgg, :], tq_half,
                                         qidx_t[:, (gt + h1_) * 8:(gt + dgg) * 8],
                                         (dgg - h1_) * P, (dgg - h1_) * P, TWD, queue_num=1)
                    nc.gpsimd.dma_gather(gk[:, 0:h1_, :], tk_half,
                                         kidx_t[:, gt * 8:(gt + h1_) * 8],
                                         h1_ * P, h1_ * P, TWD, queue_num=2)
                    nc.gpsimd.dma_gather(gk[:, h1_:dgg, :], tk_half,
                                         kidx_t[:, (gt + h1_) * 8:(gt + dgg) * 8],
                                         (dgg - h1_) * P, (dgg - h1_) * P, TWD, queue_num=3)
                    prod = wk.tile([P, DG, H], BF, tag="prod")
                    nc.vector.tensor_tensor(out=prod[:, 0:dgg, :], in0=gq[:, 0:dgg, 0:H],
                                            in1=gk[:, 0:dgg, 0:H], op=AG.mult)
                    l1 = wk.tile([P, DG * NH], F32, tag="l1")
                    nc.vector.tensor_reduce(
                        out=l1[:, 0:dgg * NH],
                        in_=prod[:, 0:dgg, :].rearrange("p g (h d) -> p (g h) d", h=NH),
                        axis=mybir.AxisListType.X, op=AG.add)
                    dlt = wk.tile([P, DG * NH], F32, tag="dlt")
                    nc.vector.tensor_tensor(
                        out=dlt[:, 0:dgg * NH].rearrange("p (g h) -> p g h", h=NH),
                        in0=l1[:, 0:dgg * NH].rearrange("p (g h) -> p g h", h=NH),
                        in1=gq[:, 0:dgg, H:H + NH], op=AG.subtract)
                    a1 = wk.tile([P, DG * NH], F32, tag="a1")
                    nc.scalar.activation(a1[:, 0:dgg * NH], dlt[:, 0:dgg * NH],
                                         mybir.ActivationFunctionType.Sigmoid)
                    ds = wk.tile([P, DG * NH], F32, tag="ds")
                    nc.vector.tensor_tensor(
                        out=ds[:, 0:dgg * NH].rearrange("p (g h) -> p g h", h=NH),
                        in0=gk[:, 0:dgg, H:H + NH],
                        in1=gq[:, 0:dgg, H + NH:H + 2 * NH], op=AG.subtract)
                    pr = wk.tile([P, DG * NH], F32, tag="pr")
                    nc.vector.tensor_tensor(out=pr[:, 0:dgg * NH], in0=a1[:, 0:dgg * NH],
                                            in1=ds[:, 0:dgg * NH], op=AG.mult)
                    prs = wk.tile([P, DG], F32, tag="prs")
                    nc.vector.tensor_reduce(
                        out=prs[:, 0:dgg],
                        in_=pr[:, 0:dgg * NH].rearrange("p (g h) -> p g h", h=NH),
                        axis=mybir.AxisListType.X, op=AG.add)
                    s0s = wk.tile([P, DG], F32, tag="s0s")
                    nc.vector.tensor_reduce(out=s0s[:, 0:dgg],
                                            in_=gq[:, 0:dgg, H + NH:H + 2 * NH],
                                            axis=mybir.AxisListType.X, op=AG.add)
                    rr = wk.tile([P, DG], F32, tag="rr")
                    nc.vector.tensor_tensor(out=rr[:, 0:dgg], in0=prs[:, 0:dgg],
                                            in1=s0s[:, 0:dgg], op=AG.add)
                    nc.scalar.activation(colbuf[:, gt:gt + dgg], rr[:, 0:dgg],
                                         mybir.ActivationFunctionType.Sigmoid,
                                         bias=bsum_t[:])
                    gt += dgg
                    done += dgg
            nc.leave_named_scope("decode", sD[0], False)

            for c0 in range(0, NDT, P):
                w = min(P, NDT - c0)
                po = ptp.tile([P, P], F32, tag="pT", space="PSUM")
                nc.tensor.transpose(po[:w, :], colbuf[:, c0:c0 + w], idf_t[:])
                ob = wk.tile([P, P], F32, tag="obx")
                nc.vector.tensor_copy(out=ob[:w, :], in_=po[:w, :])
                nc.sync.dma_start(
                    out_t[c0 * P:(c0 + w) * P].rearrange("(a b) -> a b", b=P), ob[:w, :])
    nc.compile()
    return nc


# ----------------------------------------------------------------------------
_CACHE = {}

TRACE = False
LAST_EXEC_NS = None
LAST_RESULT = None


def kernel(**inputs):
    import concourse.bass_utils as bass_utils
    global LAST_EXEC_NS, LAST_RESULT
    in_maps, meta, inv_core = build_host_data(**inputs)
    key = (meta["NPAD"], meta["T0"], meta["T1"], meta["NDT"], meta["NT"], meta["hasb1"])
    if key not in _CACHE:
        _CACHE[key] = build_program(meta)
    nc = _CACHE[key]
    trace = bool(TRACE)
    if trace:
        try:
            from trn_agent_boot.trn_boot import _ntff_profile_via_ctypes
            import antenv.axon_hooks as ah
            if ah.get_axon_ntff_profile_hook() is None:
                ah.set_axon_ntff_profile_hook(
                    _ntff_profile_via_ctypes("/opt/axon/libaxon_pjrt.so"))
        except Exception:
            trace = False
    res = bass_utils.run_bass_kernel_spmd(nc, in_maps, core_ids=list(range(NCORES)),
                                          trace=trace)
    LAST_EXEC_NS = res.exec_time_ns
    LAST_RESULT = res
    EP = meta["EP"]
    out = np.zeros(EP, np.float32)
    for c in range(NCORES):
        iv = inv_core[c]
        m = iv >= 0
        out[iv[m]] = res.results[c]["out"][m]
    return out

